# revision 36
# baseline (speedup 1.0000x reference)
"""Two-layer GCN (PyG GCNConv defaults) on 8 Trainium2 NeuronCores.

v5 strategy (graph/data parallel, merged gathers, 64-granular capacities):
  - Nodes padded to 102400 = 8 x 12800; each core owns 100 dst blocks of 128.
  - Sources bucketed into 4 interleaved quarter-shard ranges (25600 rows each,
    int16-indexable); bucket table k is produced by AllGather-ing every core's
    k-th local quarter, so the per-layer AllGather is split into 4 chunks that
    overlap with compute.
  - Gathers are merged: ONE dma_gather per (supergroup of 5 dst blocks,
    bucket) covering the 5 blocks' segments back-to-back in the idx stream
    (160 calls total vs 800). The binding resource is the Q7 SWDGE
    descriptor-generation rate (~2.6ns/gathered row, serialized on the pool
    engine), so segment capacities are 64-slot-granular (region padded to
    128) to minimize gathered rows; straddling 128-slot chunks are consumed
    by partition-sliced matmuls. Every slot is valid: padding uses idx=0
    (gathers table row 0) + selector sentinel dl=255, so no runtime valid
    counts, no reg_loads, and no memsets are needed.
  - Tables use a partition-major layout (node n -> partition n%128, column
    block n//128) so phase-A stores and own-shard (gown) loads are large
    contiguous per-partition DMAs instead of 256B strided descriptors; the
    gather row permutation is folded into the host-computed indices.
  - One is_equal selector build per (supergroup, bucket) region on DVE;
    transposed accumulation psum[feat,lane] += msg.T @ St per 128-slot chunk.
  - Self-loop via contiguous own-shard load + identity matmul; dq[dst] by a
    per-block DVE multiply; bias via scalar activation bias vector.
  - Supergroups of 5 dst blocks hold live PSUM accumulators; layer-2
    AllGather chunks fire as their quarter of g2 lands.
  - x is converted to bf16 on host (halves the phase-A input DMA).
Output is produced transposed ([D, SH] per core) and transposed back on host.
"""
import sys

sys.path.insert(0, "/opt/trn_rl_repo")

import numpy as np
import ml_dtypes

import concourse.bacc as bacc
import concourse.mybir as mybir
import concourse.tile as tile
from concourse.bass_utils import run_bass_kernel_spmd

NCORES = 8
N = 100000
E = 1600000
D = 128
SH = 12800              # dst shard per core (100 blocks of 128)
NP_ = SH * NCORES       # padded node count 102400
BLK = SH // 128         # 100 dst blocks per core
NB = 4                  # source buckets
SUB = SH // NB          # 3200 rows per local quarter (=25 blocks)
SUBBLK = SUB // 128     # 25
SG = 5                  # supergroup: blocks per PSUM residency group
NSG = BLK // SG         # 20
BKT = SUB * NCORES      # 25600 rows per bucket table
NEG = 0.01
MBUFS = 3               # in-flight gathered-message buffers per bucket tag
SBUFS = 2               # in-flight selector buffers per bucket tag

fp32 = mybir.dt.float32
bf16 = mybir.dt.bfloat16
i16 = mybir.dt.int16
i32 = mybir.dt.int32
fp8 = mybir.dt.float8e4

_CACHE = {}


def _plan_from_counts(maxcnt):
    """maxcnt: [BLK*NB] max (over cores) edges per (block, bucket) segment,
    indexed b*NB+r. Segment capacities are 64-slot-granular; each
    (supergroup, bucket) region is padded to a 128 multiple so chunk/column
    addressing stays aligned. Returns per-segment slot capacities, absolute
    per-segment slot offsets, per-region slot offsets + chunk counts, ts."""
    caps = np.maximum(64, -(-maxcnt // 64) * 64)   # [BLK*NB] slots
    seg_off = np.zeros(BLK * NB, np.int64)
    rg_off = np.zeros((NSG, NB), np.int64)
    rg_nck = np.zeros((NSG, NB), np.int64)
    pos = 0
    for sg in range(NSG):
        for r in range(NB):
            rg_off[sg, r] = pos
            rel = 0
            for b in range(sg * SG, sg * SG + SG):
                seg_off[b * NB + r] = pos + rel
                rel += int(caps[b * NB + r])
            tot = -(-rel // 128) * 128
            rg_nck[sg, r] = tot // 128
            pos += tot
    return caps, seg_off, rg_off, rg_nck, int(pos)


def _build(plan_key):
    caps, seg_off, rg_off, rg_nck, ts = plan_key
    nch = ts // 128
    ts16 = ts // 16
    nckmax = int(rg_nck.max())

    nc = bacc.Bacc("TRN2", num_devices=NCORES, num_swdge_queues=4,
                   dynamic_dma_scratch_size=65536)
    xT_in = nc.dram_tensor("xT", [128, SH], bf16, kind="ExternalInput")
    w1_in = nc.dram_tensor("w1", [128, 128], bf16, kind="ExternalInput")
    w2_in = nc.dram_tensor("w2", [128, 128], bf16, kind="ExternalInput")
    b1_in = nc.dram_tensor("b1c", [128, 1], fp32, kind="ExternalInput")
    b2_in = nc.dram_tensor("b2c", [128, 1], fp32, kind="ExternalInput")
    dqf_in = nc.dram_tensor("dqf", [128, BLK], fp32, kind="ExternalInput")
    dqb_in = nc.dram_tensor("dqb", [128, SH], fp32, kind="ExternalInput")
    idx_in = nc.dram_tensor("idx", [128, ts16], i16, kind="ExternalInput")
    dl_in = nc.dram_tensor("dl", [128, nch], bf16, kind="ExternalInput")
    iota_in = nc.dram_tensor("iota", [128, 128], bf16,
                             kind="ExternalInput")
    ident_in = nc.dram_tensor("ident", [128, 128], bf16, kind="ExternalInput")
    out_t = nc.dram_tensor("out", [128, SH], fp32, kind="ExternalOutput")

    with tile.TileContext(nc) as tc:
        with (
            tc.tile_pool(name="const", bufs=1) as cpool,
            tc.tile_pool(name="xchunk", bufs=3) as xpool,
            tc.tile_pool(name="msg", bufs=MBUFS) as mpool,
            tc.tile_pool(name="st", bufs=SBUFS) as spool,
            tc.tile_pool(name="gown", bufs=2) as gpool,
            tc.tile_pool(name="dqs", bufs=2) as qpool,
            tc.tile_pool(name="fin", bufs=6) as fpool,
            tc.tile_pool(name="ps_a", bufs=1, space="PSUM") as ps_a,
            tc.tile_pool(name="ps_w", bufs=3, space="PSUM") as ps_w,
            tc.tile_pool(name="dram", bufs=1, space="DRAM") as dram,
        ):
            # ---- small resident constants --------------------------------
            w1b = cpool.tile([128, 128], bf16)
            nc.sync.dma_start(w1b[:], w1_in[:])
            w2b = cpool.tile([128, 128], bf16)
            nc.sync.dma_start(w2b[:], w2_in[:])
            b1c = cpool.tile([128, 1], fp32)
            nc.sync.dma_start(b1c[:], b1_in[:])
            b2c = cpool.tile([128, 1], fp32)
            nc.sync.dma_start(b2c[:], b2_in[:])
            dqf = cpool.tile([128, BLK], fp32)
            nc.sync.dma_start(dqf[:], dqf_in[:])
            iota1 = cpool.tile([128, 128], bf16)
            nc.sync.dma_start(iota1[:], iota_in[:])
            ident = cpool.tile([128, 128], bf16)
            nc.sync.dma_start(ident[:], ident_in[:])

            # DRAM: per-quarter local shards (partition-major layout) +
            # gathered bucket tables (flat [BKT, D] row view of the same
            # bytes, as the gather's 256B-row address space)
            g1_b = [dram.tile([128, SUB], bf16, name=f"g1b{k}")
                    for k in range(NB)]
            g1_full = [dram.tile([BKT, D], bf16, name=f"g1f{k}",
                                 addr_space="Shared") for k in range(NB)]
            g2_b = [dram.tile([128, SUB], bf16, name=f"g2b{k}")
                    for k in range(NB)]
            g2_full = [dram.tile([BKT, D], bf16, name=f"g2f{k}",
                                 addr_space="Shared") for k in range(NB)]

            def ag(src, dst):
                nc.gpsimd.collective_compute(
                    "AllGather", mybir.AluOpType.bypass,
                    replica_groups=[list(range(NCORES))],
                    ins=[src[:].opt()], outs=[dst[:].opt()])

            # ---- phase A: g1 = dq * (x @ W1) on own shard ----------------
            APIECE = 5
            with nc.named_scope("phaseA"):
                for sub in range(NB):
                    for pc in range(SUBBLK // APIECE):
                        c0 = sub * SUBBLK + pc * APIECE
                        xcb = xpool.tile([128, APIECE * 128], bf16, tag="xcb")
                        nc.sync.dma_start(
                            xcb[:], xT_in[:, c0 * 128:(c0 + APIECE) * 128])
                        stage = xpool.tile([128, APIECE * 128], bf16,
                                           tag="ast")
                        for j in range(APIECE):
                            ph = ps_w.tile([128, 128], fp32, space="PSUM",
                                           tag="ph")
                            nc.tensor.matmul(
                                out=ph[:], lhsT=xcb[:, j * 128:(j + 1) * 128],
                                rhs=w1b[:], start=True, stop=True)
                            nc.scalar.activation(
                                stage[:, j * 128:(j + 1) * 128], ph[:],
                                mybir.ActivationFunctionType.Copy,
                                scale=dqf[:, c0 + j:c0 + j + 1])
                        a0 = pc * APIECE * 128
                        nc.sync.dma_start(
                            g1_b[sub][:, a0:a0 + APIECE * 128], stage[:])
                    # fire this quarter's layer-1 AllGather chunk as soon as
                    # its stores are issued (deps gate the actual transfer)
                    ag(g1_b[sub], g1_full[sub])
                    if sub == 0:
                        # big streams: load after the first quarter so they
                        # don't delay phase A's own DMAs
                        idxS = cpool.tile([128, ts16], i16)
                        nc.sync.dma_start(idxS[:], idx_in[:])
                        dlS = cpool.tile([128, nch], bf16)
                        nc.sync.dma_start(dlS[:], dl_in[:])

            # ---- aggregation (shared for both layers) --------------------
            def aggregate(layer, tables, gsrc, fin):
                def emit(sg, r):
                    nck = int(rg_nck[sg, r])
                    so = int(rg_off[sg, r])            # region start slot
                    msg = mpool.tile([128, nckmax, 128], bf16,
                                     tag=f"m{r}")
                    nc.gpsimd.dma_gather(
                        msg[:, 0:nck, :], tables[r][:],
                        idxS[:, so // 16:so // 16 + nck * 8],
                        nck * 128, nck * 128, 128,
                        single_packet=False, queue_num=r)
                    st = spool.tile([128, nckmax, 128], fp8,
                                    tag=f"s{r}")
                    c0 = so // 128
                    nc.vector.tensor_tensor(
                        st[:, 0:nck, :],
                        iota1[:, None, :].to_broadcast(
                            [128, nck, 128]),
                        dlS[:, c0:c0 + nck].to_broadcast(
                            [128, nck, 128]),
                        mybir.AluOpType.is_equal)
                    return msg, st

                # Staggered warm-up: emit the first PRE supergroups' gathers
                # bucket-major (r outer) so Q7 has queued work for the
                # already-AllGathered buckets while later chunks land. PRE
                # must not exceed MBUFS (buffer WAR would deadlock-chain).
                PRE = 3
                pend = {}
                for r in range(NB):
                    for sg in range(PRE):
                        pend[sg, r] = emit(sg, r)
                for sg in range(NSG):
                    a0 = (sg % 5) * SG * 128
                    gown = gpool.tile([128, SG * 128], bf16, tag="gown")
                    nc.sync.dma_start(
                        gown[:], gsrc[sg // 5][:, a0:a0 + SG * 128])
                    dqsg = qpool.tile([128, SG * 128], fp32, tag="dqs")
                    nc.sync.dma_start(
                        dqsg[:], dqb_in[:, sg * SG * 128:(sg + 1) * SG * 128])
                    acc = [ps_a.tile([128, 128], fp32, space="PSUM",
                                     tag=f"a{j}", name=f"acc{j}")
                           for j in range(SG)]
                    msgs, sts = [], []
                    for r in range(NB):
                        if (sg, r) in pend:
                            msg, st = pend.pop((sg, r))
                        else:
                            msg, st = emit(sg, r)
                        msgs.append(msg)
                        sts.append(st)
                    for r in range(NB):
                        off = 0                        # region-relative slots
                        for j in range(SG):
                            b = sg * SG + j
                            o1 = off + int(caps[b * NB + r])
                            first = True
                            for c in range(off // 128, (o1 - 1) // 128 + 1):
                                p0 = max(off - c * 128, 0)
                                p1 = min(o1 - c * 128, 128)
                                nc.tensor.matmul(
                                    out=acc[j][:],
                                    lhsT=msgs[r][p0:p1, c, :],
                                    rhs=sts[r][p0:p1, c, :],
                                    start=(r == 0 and first), stop=False)
                                first = False
                            off = o1
                    stage2 = None
                    if layer == 1:
                        stage2 = fpool.tile([128, SG * 128], bf16, tag="g2s",
                                            bufs=2)
                    for j in range(SG):
                        b = sg * SG + j
                        nc.tensor.matmul(
                            out=acc[j][:], lhsT=gown[:, j * 128:(j + 1) * 128],
                            rhs=ident[:], start=False, stop=True)
                        fin(b, acc[j], stage2, j, dqsg, j * 128)
                    if layer == 1:
                        nc.sync.dma_start(
                            g2_b[sg // 5][:, a0:a0 + SG * 128], stage2[:])
                        if sg % 5 == 4:
                            # quarter sg//5 of g2 is complete: fire its
                            # layer-2 AllGather chunk now so it overlaps the
                            # rest of layer-1 aggregation
                            ag(g2_b[sg // 5], g2_full[sg // 5])

            def fin1(b, ps, stage2, j, dqsg, q0):
                v = fpool.tile([128, 128], bf16, tag="v")
                nc.vector.tensor_tensor(
                    v[:], ps[:], dqsg[:, q0:q0 + 128],
                    mybir.AluOpType.mult)
                r1T = fpool.tile([128, 128], bf16, tag="r1T")
                nc.scalar.activation(r1T[:], v[:],
                                     mybir.ActivationFunctionType.Lrelu,
                                     bias=b1c[:, 0:1], scale=1.0, alpha=NEG)
                ph2 = ps_w.tile([128, 128], fp32, space="PSUM", tag="ph")
                nc.tensor.matmul(out=ph2[:], lhsT=r1T[:], rhs=w2b[:],
                                 start=True, stop=True)
                nc.scalar.activation(stage2[:, j * 128:(j + 1) * 128], ph2[:],
                                     mybir.ActivationFunctionType.Copy,
                                     scale=dqf[:, b:b + 1])

            def fin2(b, ps, stage2, j, dqsg, q0):
                v = fpool.tile([128, 128], bf16, tag="v")
                nc.vector.tensor_tensor(
                    v[:], ps[:], dqsg[:, q0:q0 + 128],
                    mybir.AluOpType.mult)
                ob = fpool.tile([128, 128], fp32, tag="ob")
                nc.scalar.activation(ob[:], v[:],
                                     mybir.ActivationFunctionType.Lrelu,
                                     bias=b2c[:, 0:1], scale=1.0, alpha=NEG)
                nc.sync.dma_start(out_t[:, b * 128:(b + 1) * 128], ob[:])

            with nc.named_scope("agg1"):
                aggregate(1, g1_full, g1_b, fin1)

            with nc.named_scope("agg2"):
                aggregate(2, g2_full, g2_b, fin2)

    nc.compile()
    return nc


def _preprocess(x, edge_index):
    src = np.asarray(edge_index[0], dtype=np.int64)
    dst = np.asarray(edge_index[1], dtype=np.int64)

    deg = np.bincount(dst, minlength=NP_).astype(np.float64) + 1.0
    dq = (1.0 / np.sqrt(deg)).astype(np.float32)

    core = dst // SH
    b = (dst % SH) // 128
    lane = (dst % 128).astype(np.float32)
    r = (src % SH) // SUB
    q = (src % SH) - r * SUB                      # position within quarter
    # partition-major table layout: local row = (q%128)*SUBBLK + q//128
    srcloc = ((src // SH) * SUB + (q % 128) * SUBBLK + q // 128).astype(
        np.int16)

    segid = b * NB + r
    seg_global = core * (NB * BLK) + segid
    counts = np.bincount(seg_global, minlength=NCORES * NB * BLK) \
        .reshape(NCORES, NB * BLK)
    maxcnt = counts.max(axis=0)
    caps, seg_off, rg_off, rg_nck, ts = _plan_from_counts(maxcnt)

    # slot assignment
    order = np.argsort(seg_global, kind="stable")
    ks = seg_global[order]
    starts = np.zeros(NCORES * NB * BLK + 1, np.int64)
    np.cumsum(np.bincount(seg_global, minlength=NCORES * NB * BLK),
              out=starts[1:])
    pos = np.arange(E, dtype=np.int64) - starts[ks]
    slot = seg_off[segid[order]] + pos

    idx_arr = np.zeros((NCORES, ts), np.int16)     # padding gathers row 0
    idx_arr[core[order], slot] = srcloc[order]
    dl_arr = np.full((NCORES, ts), 255.0, np.float32)
    dl_arr[core[order], slot] = lane[order]

    xpad = np.zeros((NP_, D), np.float32)
    xpad[:N] = x

    iota = np.tile(np.arange(128, dtype=np.float32), (128, 1)) \
        .astype(ml_dtypes.bfloat16)
    ident = np.eye(128, dtype=ml_dtypes.bfloat16)

    return (caps, seg_off, rg_off, rg_nck, ts), idx_arr, dl_arr, dq, xpad, \
        iota, ident


def kernel(x, W1, b1, W2, b2, edge_index, batch):
    x = np.asarray(x, np.float32)
    W1 = np.asarray(W1, np.float32)
    W2 = np.asarray(W2, np.float32)
    b1 = np.asarray(b1, np.float32)
    b2 = np.asarray(b2, np.float32)

    plan, idx_arr, dl_arr, dq, xpad, iota, ident = _preprocess(x, edge_index)

    key = (tuple(plan[0].tolist()), plan[4])
    if key not in _CACHE:
        _CACHE[key] = _build(plan)
    nc = _CACHE[key]

    in_maps = []
    for c in range(NCORES):
        sl = slice(c * SH, (c + 1) * SH)
        dqc = dq[sl]
        wrapped = np.tile(idx_arr[c].reshape(-1, 16).T, (8, 1))
        in_maps.append({
            "xT": np.ascontiguousarray(xpad[sl].T).astype(ml_dtypes.bfloat16),
            "w1": W1.astype(ml_dtypes.bfloat16),
            "w2": W2.astype(ml_dtypes.bfloat16),
            "b1c": np.ascontiguousarray(b1[:, None]),
            "b2c": np.ascontiguousarray(b2[:, None]),
            "dqf": np.ascontiguousarray(dqc.reshape(BLK, 128).T),
            "dqb": np.ascontiguousarray(np.tile(dqc[None, :], (128, 1))),
            "idx": np.ascontiguousarray(wrapped),
            "dl": np.ascontiguousarray(
                dl_arr[c].reshape(-1, 128).T.astype(ml_dtypes.bfloat16)),
            "iota": iota, "ident": ident,
        })

    import os
    trace = bool(os.environ.get("KERNEL_TRACE"))
    rr = run_bass_kernel_spmd(nc, in_maps, list(range(NCORES)), trace=trace)
    if trace:
        kernel.last_results = rr
    out = np.concatenate(
        [rr.results[c]["out"].T for c in range(NCORES)], axis=0)
    return np.ascontiguousarray(out[:N])
